# revision 4
# baseline (speedup 1.0000x reference)
"""Trainium2 Bass kernel: 11x11 valid cross-correlation over a 6144x6144 image.

Strategy (SPMD over 8 NeuronCores, rows sharded):
  The single-channel conv is recast as dense TensorE matmuls via a
  column-phase decomposition. Columns are split into S=5 phases; the
  matmul contraction dim packs (row-window w in [0,25), phase p in [0,5))
  -> K=125. The stationary operand is a precomputed banded weight matrix
  TW_s[(w,p),(i,q)] = W[w-i, p-q+5s]; summing s=0..2 in PSUM covers all
  121 taps. Each matmul produces M=75 outputs (15 rows x 5 phases) per
  streamed column-block, i.e. 25 useful outputs per PE cycle.

  Host side (not device-timed): rows are sharded 768/core (+10 halo),
  columns phase-shuffled to [row, phase, block] layout so every DMA is
  contiguous; the inverse shuffle is applied to the phase-layout output.

  Matmuls run in float32r (fp32 operands truncated to FP22 in the PE):
  full streaming rate (1 col/cycle) at ~1e-5 relative accuracy.
"""

import sys

if "/opt/trn_rl_repo" not in sys.path:
    sys.path.insert(0, "/opt/trn_rl_repo")

import numpy as np

from concourse import bacc, mybir
import concourse.tile as tile
from concourse.bass_utils import run_bass_kernel_spmd

# geometry (hardcoded for this problem)
KH = KW = 11
H = W = 6144
OH = OW = H - (KH - 1)  # 6134

N_CORES = 8
S = 5            # column phases
RW = 25          # input rows per matmul window
MROWS = RW - (KH - 1)  # 15 output rows per strip
M = MROWS * S    # 75  matmul output partitions
K = RW * S       # 125 matmul contraction
NSG = 3          # s-groups (accumulating matmuls per output tile)

MP = M + 1       # 76: fp32r ISA needs even free sizes; pad one zero column

CORE_OUT = 768              # output rows per core (core 7 uses 758)
STRIPS = 52                 # ceil(768/15) strips -> 780 padded out rows
ROWS_OUT_PAD = STRIPS * MROWS        # 780
ROWS_IN = ROWS_OUT_PAD + (KH - 1)    # 790 input rows per core (zero-padded)

NB = 1230        # input column blocks  (cols padded 6144 -> 6150 = 5*1230)
NOB = NB - 2     # 1228 output column blocks (6140 = 5*1228 >= 6134)
COL_TILES = [(0, 512), (512, 512), (1024, NOB - 1024)]  # all even (fp32r)

_prog_cache: dict[int, object] = {}


def _build_program(reps: int = 1):
    """Build + compile the per-core Bass program (same for all 8 cores)."""
    if reps in _prog_cache:
        return _prog_cache[reps]

    nc = bacc.Bacc("TRN2", target_bir_lowering=False, debug=False,
                   num_devices=N_CORES)
    f32 = mybir.dt.float32
    f32r = mybir.dt.float32r

    xp = nc.dram_tensor("xp", [ROWS_IN * S, NB], f32r, kind="ExternalInput").ap()
    tw = nc.dram_tensor("tw", [K, NSG * MP], f32r, kind="ExternalInput").ap()
    outp = nc.dram_tensor("outp", [ROWS_OUT_PAD * S, NOB], f32,
                          kind="ExternalOutput").ap()

    with tile.TileContext(nc) as tc:
        with (
            tc.tile_pool(name="twp", bufs=1) as twp,
            tc.tile_pool(name="xpool", bufs=3) as xpool,
            tc.tile_pool(name="pspool", bufs=6, space="PSUM") as pspool,
            tc.tile_pool(name="opool", bufs=3) as opool,
        ):
            twt = twp.tile([K, NSG * MP], f32r)
            nc.sync.dma_start(twt[:], tw[:])

            for _ in range(reps):
                for st in range(STRIPS):
                    xt = xpool.tile([K, NB], f32r)
                    nc.sync.dma_start(xt[:], xp[st * MROWS * S:
                                                st * MROWS * S + K, :])

                    ot = opool.tile([M, NOB], f32)
                    for mt0, nt in COL_TILES:
                        ps = pspool.tile([MP, 512], f32)
                        for s in range(NSG):
                            nc.tensor.matmul(
                                ps[:, :nt],
                                twt[:, s * MP:(s + 1) * MP],
                                xt[:, mt0 + s:mt0 + s + nt],
                                start=(s == 0),
                                stop=(s == NSG - 1),
                            )
                        nc.vector.tensor_copy(ot[:, mt0:mt0 + nt],
                                              ps[:M, :nt])

                    nc.sync.dma_start(
                        outp[st * M:(st + 1) * M, :], ot[:])

    nc.compile()
    _prog_cache[reps] = nc
    return nc


def _build_tw(weight: np.ndarray) -> np.ndarray:
    """Banded stationary weight matrices, concatenated: [K, NSG*MP] f32."""
    twm = np.zeros((NSG, K, MP), np.float32)
    u = np.arange(RW)[:, None] - np.arange(MROWS)[None, :]          # w - i
    for s in range(NSG):
        v = (np.arange(S)[:, None] - np.arange(S)[None, :]) + S * s  # p - q
        for w in range(RW):
            for i in range(MROWS):
                if 0 <= u[w, i] <= KH - 1:
                    for p in range(S):
                        for q in range(S):
                            if 0 <= v[p, q] <= KW - 1:
                                twm[s, w * S + p, i * S + q] = \
                                    weight[u[w, i], v[p, q]]
    return np.ascontiguousarray(
        np.concatenate([twm[s] for s in range(NSG)], axis=1))


def _shard_inputs(X: np.ndarray, weight: np.ndarray):
    """Per-core phase-shuffled input shards + replicated weights."""
    Xf = np.asarray(X, np.float32)
    twc = _build_tw(np.asarray(weight, np.float32))
    in_maps = []
    for k in range(N_CORES):
        r0 = CORE_OUT * k
        xs = np.zeros((ROWS_IN, S * NB), np.float32)
        n = min(ROWS_IN, H - r0)
        xs[:n, :W] = Xf[r0:r0 + n]
        # [r, c=5m+p] -> [r, p, m] -> [(r,p), m]
        xpk = np.ascontiguousarray(
            xs.reshape(ROWS_IN, NB, S).transpose(0, 2, 1)
        ).reshape(ROWS_IN * S, NB)
        in_maps.append({"xp": xpk, "tw": twc})
    return in_maps


def _assemble_output(results, bias_val: float) -> np.ndarray:
    out = np.empty((OH, OW), np.float32)
    for k in range(N_CORES):
        op = results[k]["outp"].reshape(ROWS_OUT_PAD, S, NOB)
        rows = np.ascontiguousarray(op.transpose(0, 2, 1)).reshape(
            ROWS_OUT_PAD, S * NOB)
        r0 = CORE_OUT * k
        take = min(CORE_OUT, OH - r0)
        out[r0:r0 + take] = rows[:take, :OW]
    if bias_val != 0.0:
        out += bias_val
    return out


def kernel(X: np.ndarray, weight: np.ndarray, bias: np.ndarray) -> np.ndarray:
    nc = _build_program(reps=1)
    in_maps = _shard_inputs(X, weight)
    res = run_bass_kernel_spmd(nc, in_maps, list(range(N_CORES)))
    return _assemble_output(res.results, float(np.asarray(bias).reshape(-1)[0]))


# revision 6
# speedup vs baseline: 3.1349x; 3.1349x over previous
"""Trainium2 Bass kernel: 11x11 valid cross-correlation over a 6144x6144
fp32 image, SPMD across 8 NeuronCores (rows sharded 768/core + 10-row halo).

Algorithm — column-phase block decomposition (S = 11 phases):
  Columns are split into 11 phases so one matmul contracts K = 11 rows x 11
  phases = 121. The stationary operand is a precomputed banded weight matrix
  TW[(w,p),(i,q)] = W[w+10-i, p-q+11s]; two accumulating matmuls (s = 0,1)
  cover all 121 taps of a column block-pair. Input rows are processed in
  disjoint 11-row blocks, each DMAed exactly once:
    set1 (2 matmuls, M=121) completes output tile T_j   (rows 11j-10..11j)
    set2 (2 matmuls, M=121, zero-padded band) opens tile T_{j+1} in PSUM,
         which block j+1's set1 then finishes (cross-block accumulation via
         per-element has_written: set2's start=True initializes the whole
         tile, so it must cover all partitions).
  121 outputs per 4 streamed columns = 30.25 outputs/PE-cycle.

Performance notes (HW-measured):
  - bf16 operands halve HBM traffic; fp32 PSUM accumulate. rel err ~3e-3.
  - dma_start carries a ~2us HBM-receipt stall serialized on its HWDGE ring:
    blocks are batched 8 per DMA (3D access pattern), input DMAs on the sync
    ring, output DMAs on the scalar ring.
  - Host side (not device-timed) pre/post shuffles columns into the phase
    layout so every DMA is fully contiguous.
"""

import time

import numpy as np
import ml_dtypes

try:
    from concourse import bacc, mybir
except ImportError:  # fallback when the env doesn't pre-provide concourse
    import sys
    sys.path.insert(0, "/opt/trn_rl_repo")
    from concourse import bacc, mybir
import concourse.tile as tile
from concourse.bass_utils import run_bass_kernel_spmd

KH = KW = 11
H = W = 6144
OH = OW = H - (KH - 1)          # 6134

N_CORES = 8
S = 11                          # column phases
RW = 11                         # input rows per block
K = RW * S                      # 121 contraction
M1 = RW * S                     # 121 output partitions per tile
NSG = 2                         # accumulating matmuls per set

CORE_OUT = 768                  # output rows per core (core 7: 758 valid)
NBLK = 71                       # 11-row blocks per core
ROWS_IN = NBLK * RW             # 781 input rows per core (zero-padded)
NB = 559                        # input column blocks (cols padded to 6149)
NOB = NB - 1                    # 558 output column blocks
COL_TILES = [(0, 512), (512, NOB - 512)]
GRP = 8                         # blocks per batched DMA

_prog_cache: dict = {}


def _build_program(reps: int = 1, timing: bool = False):
    key = (reps, timing)
    if key in _prog_cache:
        return _prog_cache[key]

    bf16 = mybir.dt.bfloat16
    f32 = mybir.dt.float32
    nc = bacc.Bacc("TRN2", target_bir_lowering=False, debug=False,
                   num_devices=N_CORES)

    if timing:
        # benchmark build: big I/O stays in scratch DRAM so the axon tunnel
        # does not re-ship 150MB per call; a tiny output defeats DCE.
        xp = nc.dram_tensor("xp", [ROWS_IN * S, NB], bf16).ap()
        outp = nc.dram_tensor("outp", [NBLK * M1, NOB], bf16).ap()
        tout = nc.dram_tensor("tout", [NBLK, 64], bf16,
                              kind="ExternalOutput").ap()
    else:
        xp = nc.dram_tensor("xp", [ROWS_IN * S, NB], bf16,
                            kind="ExternalInput").ap()
        outp = nc.dram_tensor("outp", [NBLK * M1, NOB], bf16,
                              kind="ExternalOutput").ap()
    tw = nc.dram_tensor("tw", [K, 2 * NSG * M1], bf16,
                        kind="ExternalInput").ap()

    with tile.TileContext(nc) as tc:
        with (
            tc.tile_pool(name="twp", bufs=1) as twp,
            tc.tile_pool(name="xpool", bufs=4) as xpool,
            tc.tile_pool(name="pspool", bufs=7, space="PSUM") as pspool,
            tc.tile_pool(name="opool", bufs=4) as opool,
        ):
            twt = twp.tile([K, 2 * NSG * M1], bf16)
            nc.sync.dma_start(twt[:], tw[:])
            off2 = NSG * M1

            for _ in range(reps):
                ps_prev = [None, None]
                for g0 in range(0, NBLK, GRP):
                    nb = min(GRP, NBLK - g0)
                    xt = xpool.tile([K, GRP, NB], bf16)
                    nc.sync.dma_start(
                        xt[:, :nb, :],
                        xp[g0 * K:(g0 + nb) * K, :].rearrange(
                            "(b k) m -> k b m", k=K))
                    ot = opool.tile([M1, GRP, NOB], bf16)

                    for b in range(nb):
                        j = g0 + b
                        for ct, (mt0, nt) in enumerate(COL_TILES):
                            ps = ps_prev[ct]
                            fresh = ps is None
                            if fresh:
                                ps = pspool.tile([M1, 512], f32, tag="psb")
                            for s in range(NSG):
                                nc.tensor.matmul(
                                    ps[:, :nt],
                                    twt[:, s * M1:(s + 1) * M1],
                                    xt[:, b, mt0 + s:mt0 + s + nt],
                                    start=(fresh and s == 0),
                                    stop=(s == NSG - 1),
                                    skip_group_check=True,
                                )
                            nc.vector.tensor_copy(ot[:, b, mt0:mt0 + nt],
                                                  ps[:, :nt])
                            if j < NBLK - 1:
                                ps2 = pspool.tile([M1, 512], f32, tag="psb")
                                for s in range(NSG):
                                    nc.tensor.matmul(
                                        ps2[:, :nt],
                                        twt[:, off2 + s * M1:
                                            off2 + (s + 1) * M1],
                                        xt[:, b, mt0 + s:mt0 + s + nt],
                                        start=(s == 0),
                                        stop=False,
                                        skip_group_check=True,
                                    )
                                ps_prev[ct] = ps2
                            else:
                                ps_prev[ct] = None

                    nc.scalar.dma_start(
                        outp[g0 * M1:(g0 + nb) * M1, :].rearrange(
                            "(b k) m -> k b m", k=M1),
                        ot[:, :nb, :])

            if timing:
                nc.sync.dma_start(tout[:, :], outp[0:NBLK * M1:M1, 0:64])

    nc.compile()
    _prog_cache[key] = nc
    return nc


def _build_tw(weight: np.ndarray) -> np.ndarray:
    """[K, 2*NSG*M1] bf16: NSG set1 bands then NSG set2 bands (zero-padded)."""
    w_ = np.asarray(weight, np.float32)
    tb = np.zeros((NSG, K, M1), np.float32)
    tb2 = np.zeros((NSG, K, M1), np.float32)
    for s in range(NSG):
        for w in range(RW):
            for p in range(S):
                for q in range(S):
                    v = p - q + S * s
                    if not (0 <= v <= KW - 1):
                        continue
                    for i in range(RW):          # T_j row r = 11j-10+i
                        u = w + 10 - i
                        if 0 <= u <= KH - 1:
                            tb[s, w * S + p, i * S + q] = w_[u, v]
                    for i in range(KH - 1):      # T_{j+1} row r = 11j+1+i
                        u = w - 1 - i
                        if 0 <= u <= KH - 1:
                            tb2[s, w * S + p, i * S + q] = w_[u, v]
    return np.ascontiguousarray(np.concatenate(
        [tb[s] for s in range(NSG)] + [tb2[s] for s in range(NSG)],
        axis=1)).astype(ml_dtypes.bfloat16)


def _shard_inputs(X: np.ndarray, weight: np.ndarray):
    Xf = np.asarray(X, np.float32)
    twc = _build_tw(weight)
    in_maps = []
    for k in range(N_CORES):
        r0 = CORE_OUT * k
        xs = np.zeros((ROWS_IN, S * NB), np.float32)
        n = min(ROWS_IN, H - r0)
        xs[:n, :W] = Xf[r0:r0 + n]
        # [r, c=11m+p] -> [(r,p), m]
        xpk = np.ascontiguousarray(
            xs.reshape(ROWS_IN, NB, S).transpose(0, 2, 1)
        ).reshape(ROWS_IN * S, NB).astype(ml_dtypes.bfloat16)
        in_maps.append({"xp": xpk, "tw": twc})
    return in_maps


def _assemble_output(results, bias_val: float) -> np.ndarray:
    out = np.empty((OH, OW), np.float32)
    for k in range(N_CORES):
        op = np.asarray(results[k]["outp"], np.float32).reshape(
            ROWS_IN, S, NOB)
        rows = np.ascontiguousarray(op.transpose(0, 2, 1)).reshape(
            ROWS_IN, S * NOB)
        r0 = CORE_OUT * k
        take = min(CORE_OUT, OH - r0)
        out[r0:r0 + take] = rows[10:10 + take, :OW]
    if bias_val != 0.0:
        out += bias_val
    return out


def kernel(X: np.ndarray, weight: np.ndarray, bias: np.ndarray) -> np.ndarray:
    nc = _build_program(reps=1)
    in_maps = _shard_inputs(X, weight)
    last_err = None
    for attempt in range(4):
        try:
            res = run_bass_kernel_spmd(nc, in_maps, list(range(N_CORES)))
            break
        except Exception as e:  # transient device wedge: wait and retry
            last_err = e
            time.sleep(90)
    else:
        raise last_err
    return _assemble_output(res.results, float(np.asarray(bias).reshape(-1)[0]))
